# revision 17
# baseline (speedup 1.0000x reference)
"""Trainium2 Bass kernel for nn_BaselineOut (article/option additive-attention MRC head).

Contract: kernel(**inputs) takes FULL unsharded inputs (numpy), returns FULL
[32, 5] float32 logits.  Internally: data-parallel over batch across 8 cores
(4 batch items per core), all params replicated.

Math notes (vs reference):
  - oqc gather is done as a one-hot matmul on device (host only encodes the
    int indices as a one-hot matrix - a layout/encoding transform).
  - V-projection is pulled out of the attention sum by linearity:
        sum_l softmax_l * (V @ Vw^T + Vb) = (sum_l softmax_l * V) @ Vw^T + Vb
    so the [B*L,H]x[H,H] V matmul collapses to a weighted sum over L plus a
    tiny [B,H]x[H,H] matmul.
  - Consecutive linear maps with no nonlinearity between are constant-folded
    on host (weight-weight products):
      * aq -> Qp_d: one matmul with Wqv = d_Qw @ a_Vw^T and a folded bias.
      * feats -> logits: per-option folded weights Ff_o = d_Vw^T @ f_w[:,o]^T.
  - softmax logit bias (vb) is dropped: softmax is shift-invariant.
  - exp is computed without max-subtraction: |logit| <= ||vw||_1 ~ 36, well
    inside fp32 exp range.
  - The article branch (Q/K projections, tanh dot, weighted sum) runs in
    fp8e4 with DoubleRow matmuls (2 k-chunks per instruction).  This is safe:
    the article attention output u only reaches the logits as a small
    additive query shift (std ~0.07) inside the option tanh (argument std
    ~1.4), so multi-percent noise in u damps to ~0.3% at the output.  The
    option branch K-projection stays bf16-or-better since its noise
    propagates ~1:1 into the logits.
  - The weighted V-sum is one fused DVE pass per h-chunk:
    scalar_tensor_tensor(T * scores_psum) with accum_out.
  - The option-branch normalization 1/sum is folded into the replicated
    score tile (sdn), and the final linear runs on the PE over the
    score-scaled option tile, producing logits^T; the host un-transposes.
  - Big tensors are host-swizzled so each SBUF partition's data is one long
    contiguous DRAM run (SWDGE throughput ~ run_length; wq/wk are fused into
    one 16KB-run tensor).  fp8/bf16 casting happens on host.  f32r must
    never be a DRAM I/O dtype (crashes NRT) - the one f32->f32r cast (ones)
    rides SWDGE.
  - The option K-projection (kpd) matmuls are issued before the uT/biasO
    chain and staged to SBUF via vector copies, so the PE fills the
    article->options dependency gap instead of idling.
"""

import functools
import sys

import numpy as np

sys.path.insert(0, "/opt/trn_rl_repo")

import ml_dtypes  # noqa: E402

import concourse.bass as bass  # noqa: E402
from concourse import bacc  # noqa: E402
import concourse.tile as tile  # noqa: E402
from concourse import mybir  # noqa: E402
from concourse.bass import ds, ts  # noqa: E402

B, LA, LQ, LO, H, OUT = 32, 2048, 64, 32, 1024, 5
NCORES = 8
BL = B // NCORES  # 4 batch items per core
NOPT = 5
F32 = mybir.dt.float32
F32R = mybir.dt.float32r
F8 = mybir.dt.float8e4
BF16 = mybir.dt.bfloat16
LT = 512  # article l-tile (free dim of the big matmuls)
NLT = LA // LT  # 4
C = H // 128  # 8 h-chunks
C2 = C // 2  # 4 h-chunk pairs (DoubleRow)
BO = BL * NOPT  # 20 (b, option) pairs per core
AF = mybir.ActivationFunctionType
ALU = mybir.AluOpType
AX = mybir.AxisListType
DR = mybir.MatmulPerfMode.DoubleRow
OUTP = 8  # final-linear out dim padded even
HALF = 2 * NOPT * LO  # 320 option columns (2 batch items)
BOH = BO // 2  # 10 (b,o) pairs per half
NP_F8 = ml_dtypes.float8_e4m3
NP_BF16 = ml_dtypes.bfloat16


def build_nc() -> bass.Bass:
    nc = bacc.Bacc("TRN2", target_bir_lowering=False, debug=False)

    # ---- DRAM I/O (per-core shard; names are the in_map keys) ----
    # Big tensors are pre-swizzled on host to [128(partition), ...contiguous].
    artT = nc.dram_tensor("artT", [BL, 128, C, LA], F8, kind="ExternalInput").ap()
    optT = nc.dram_tensor(
        "optT", [BL, 128, C, NOPT, LO], BF16, kind="ExternalInput"
    ).ap()
    qcd = nc.dram_tensor("qc", [BL, LQ, H], F8, kind="ExternalInput").ap()
    ohd = nc.dram_tensor("oh", [LQ, BL], F8, kind="ExternalInput").ap()
    wqkd = nc.dram_tensor("wqk", [128, 2, C, H], F8, kind="ExternalInput").ap()
    wQV = nc.dram_tensor("qvwT", [128, C, H], BF16, kind="ExternalInput").ap()
    wKd = nc.dram_tensor("dKwT", [128, C, H], BF16, kind="ExternalInput").ap()
    vwad = nc.dram_tensor("vwaT", [128, C2, 2, 16], F8, kind="ExternalInput").ap()
    vwdd = nc.dram_tensor("vwdT", [128, C], BF16, kind="ExternalInput").ap()
    qkbd = nc.dram_tensor("qkbT", [128, C], F32, kind="ExternalInput").ap()
    qvbd = nc.dram_tensor("qvbT", [128, C], F32, kind="ExternalInput").ap()
    fwd = nc.dram_tensor("fwT", [128, NOPT, C, OUTP], BF16, kind="ExternalInput").ap()
    fbd = nc.dram_tensor("fb", [OUTP, 1], F32, kind="ExternalInput").ap()
    onesd = nc.dram_tensor("ones1", [1, 128], F32, kind="ExternalInput").ap()
    outd = nc.dram_tensor("out", [OUT, BL], F32, kind="ExternalOutput").ap()

    with (
        tile.TileContext(nc) as tc,
        nc.allow_low_precision(reason="fp8/bf16 article branch; PE accums fp32"),
    ):
        with (
            tc.tile_pool(name="stream", bufs=1) as stream,
            tc.tile_pool(name="art", bufs=2) as artp,
            tc.tile_pool(name="wbig", bufs=1) as wbig,
            tc.tile_pool(name="mpool", bufs=3) as mpool,
            tc.tile_pool(name="spool", bufs=2) as spool,
            tc.tile_pool(name="rdpool", bufs=1) as rdpool,
            tc.tile_pool(name="ubuf", bufs=2) as ubuf,
            tc.tile_pool(name="scratch", bufs=1) as scratch,
            tc.tile_pool(name="one", bufs=1) as one,
            tc.tile_pool(name="pacc", bufs=3, space="PSUM") as pacc,
            tc.tile_pool(name="pprep", bufs=2, space="PSUM") as pprep,
            tc.tile_pool(name="prow", bufs=2, space="PSUM") as prow,
            tc.tile_pool(name="psml", bufs=1, space="PSUM") as psml,
        ):
            # ---------- split the two head-critical 2.1MB loads across both
            # DGE paths: SWDGE descriptor generation costs ~90ns/16KB packet,
            # so serializing wqk+art0 on one queue costs ~25us.  art0 rides
            # HWDGE (hardware descriptor gen) while wqk leads SWDGE.
            art = [artp.tile([128, C, LA], F8, tag="art", name=f"art{b}")
                   for b in range(BL)]
            nc.sync.dma_start(out=art[0], in_=artT[0])
            wqk = wbig.tile([128, 2, C, H], F8, tag="wqk")
            nc.gpsimd.dma_start(out=wqk, in_=wqkd)
            wq = wqk[:, 0]
            wk = wqk[:, 1]
            qct = stream.tile([LQ, BL, H], F8, tag="qct")
            for b in range(BL):
                nc.gpsimd.dma_start(out=qct[:, b, :], in_=qcd[b])
            oht = one.tile([LQ, BL], F8, tag="oht")
            nc.gpsimd.dma_start(out=oht, in_=ohd)
            ones = one.tile([1, 128], F32R, tag="ones")
            nc.gpsimd.dma_start(out=ones, in_=onesd)
            nc.gpsimd.dma_start(out=art[1], in_=artT[1])

            # ---------- HWDGE (sync): consts + option-branch weights ----------
            # v-dot weights, padded to a 16B k-pair stride (dual-fp8
            # ldweights ISA restriction); column 0 is the real vw value.
            vwa = one.tile([128, C2, 2, 16], F8, tag="vwa")
            nc.sync.dma_start(out=vwa, in_=vwad)
            vwd = one.tile([128, C], BF16, tag="vwd")
            nc.sync.dma_start(out=vwd, in_=vwdd)
            qkb = one.tile([128, C], F32, tag="qkb")
            nc.sync.dma_start(out=qkb, in_=qkbd)
            qvb = one.tile([128, C], F32, tag="qvb")
            nc.sync.dma_start(out=qvb, in_=qvbd)
            fw = one.tile([128, NOPT, C, OUTP], BF16, tag="fw")
            nc.sync.dma_start(out=fw, in_=fwd)
            fb = one.tile([OUTP, 1], F32, tag="fb")
            nc.sync.dma_start(out=fb, in_=fbd)
            wqv = wbig.tile([128, C, H], BF16, tag="w", bufs=2)
            wdk = wbig.tile([128, C, H], BF16, tag="w", bufs=2)
            nc.sync.dma_start(out=wqv, in_=wQV)
            nc.sync.dma_start(out=wdk, in_=wKd)
            OT = stream.tile([128, C, BL, NOPT, LO], BF16, tag="ot")
            for b in range(BL):
                nc.sync.dma_start(out=OT[:, :, b], in_=optT[b])

            # ---------- gather oqc via one-hot matmul ----------
            oqcT = one.tile([128, C, BL], F8, tag="oqcT")
            for c in range(C):
                po = pacc.tile([128, BL], F32, tag="acc")
                for b in range(BL):
                    nc.tensor.matmul(
                        po[:, b : b + 1],
                        lhsT=qct[:, b, ts(c, 128)],
                        rhs=oht[:, b : b + 1],
                        start=True,
                        stop=True,
                    )
                nc.vector.tensor_copy(oqcT[:, c, :], po)

            # ---------- Qp^T = aQw @ oqc^T ; article tanh bias ----------
            biasA = one.tile([128, C, BL], F32, tag="biasA")
            for co in range(C):
                pq = pacc.tile([128, BL], F32, tag="acc")
                for ci in range(C):
                    nc.tensor.matmul(
                        pq,
                        lhsT=wq[:, ci, ts(co, 128)],
                        rhs=oqcT[:, ci, :],
                        start=(ci == 0),
                        stop=(ci == C - 1),
                    )
                nc.vector.tensor_scalar_add(biasA[:, co, :], pq, qkb[:, co : co + 1])

            # ---------- article branch ----------
            # Per (b, lt): fp8 DoubleRow K-projection -> tanh (+bias) -> fp8
            # DoubleRow v-dot -> exp -> PE-replicated scores -> fused
            # multiply+reduce weighted V-sum on DVE.
            s_sums = one.tile([1, BL, NLT], F32, tag="s_sums")
            uTun = one.tile([128, C, BL], F32, tag="uTun")
            for b in range(BL):
                if b + 2 <= BL - 1:
                    nc.gpsimd.dma_start(out=art[b + 2], in_=artT[b + 2])
                T = art[b]
                upart = ubuf.tile([128, C, NLT], F32, tag="upart")
                for lt in range(NLT):
                    lg = prow.tile([2, LT], F32, tag="lg")
                    for cop in range(C2):
                        mt2 = mpool.tile([128, 2, LT], F8, tag="mt")
                        for half in range(2):
                            co = 2 * cop + half
                            kp = pacc.tile([128, LT], F32, tag="acc")
                            for ci2 in range(C2):
                                nc.tensor.matmul(
                                    kp,
                                    lhsT=wk[:, 2 * ci2 : 2 * ci2 + 2, ts(co, 128)],
                                    rhs=T[:, 2 * ci2 : 2 * ci2 + 2, ds(lt * LT, LT)],
                                    start=(ci2 == 0),
                                    stop=(ci2 == C2 - 1),
                                    perf_mode=DR,
                                )
                            nc.scalar.activation(
                                mt2[:, half, :], kp, AF.Tanh,
                                bias=biasA[:, co, b : b + 1],
                            )
                        nc.tensor.matmul(
                            lg,
                            lhsT=vwa[:, cop, :, 0:2],
                            rhs=mt2,
                            start=(cop == 0),
                            stop=(cop == C2 - 1),
                            perf_mode=DR,
                        )
                    st = spool.tile([1, LT], F32R, tag="st")
                    nc.scalar.activation(
                        st, lg[0:1, :], AF.Exp, accum_out=s_sums[:, b, lt : lt + 1]
                    )
                    # replicate s~ across partitions: ones^T (x) st via PE
                    prep = pprep.tile([128, LT], F32, tag="prep")
                    nc.tensor.matmul(prep, lhsT=ones, rhs=st, start=True, stop=True)
                    # fused weighted V-sum: upart[:,c,lt] = sum_l T*s, one pass
                    for c in range(C):
                        scr = scratch.tile([128, LT], F32, tag="scr")
                        nc.vector.scalar_tensor_tensor(
                            scr,
                            T[:, c, ds(lt * LT, LT)],
                            1.0,
                            prep,
                            op0=ALU.mult,
                            op1=ALU.mult,
                            accum_out=upart[:, c, lt : lt + 1],
                        )
                # sum the NLT partial weighted sums -> unnormalized u^T
                nc.vector.tensor_reduce(
                    uTun[:, :, b : b + 1], upart, axis=AX.X, op=ALU.add
                )

            # ---------- options K-projection, issued before the biasO chain
            # so the PE fills the article->options dependency gap; results
            # staged to SBUF via vector copies (PSUM ring stays small).
            kpds = stream.tile([128, C, 2, HALF], F32, tag="kpds")
            for co in range(C):
                for h in range(2):
                    kpd = pacc.tile([128, HALF], F32, tag="acc")
                    for ci in range(C):
                        nc.tensor.matmul(
                            kpd,
                            lhsT=wdk[:, ci, ts(co, 128)],
                            rhs=OT[:, ci, ds(2 * h, 2)],
                            start=(ci == 0),
                            stop=(ci == C - 1),
                        )
                    nc.vector.tensor_copy(kpds[:, co, h], kpd)

            # normalization factors: 1/sum(exp) per b, replicated to 128 parts
            ssb = one.tile([1, BL], F32, tag="ssb")
            nc.vector.tensor_reduce(ssb, s_sums, axis=AX.X, op=ALU.add)
            psb = psml.tile([128, BL], F32, tag="sml")
            nc.tensor.matmul(
                psb, lhsT=ones.bitcast(F32), rhs=ssb, start=True, stop=True
            )
            rs_rep = one.tile([128, BL], F32, tag="rs_rep")
            nc.vector.reciprocal(rs_rep, psb)

            uT = one.tile([128, C, BL], BF16, tag="uT")
            for b in range(BL):
                nc.vector.tensor_scalar_mul(
                    uT[:, :, b], uTun[:, :, b], rs_rep[:, b : b + 1]
                )

            # ---------- option tanh bias via folded Wqv = d_Qw a_Vw^T ----------
            biasO = one.tile([128, C, BL], F32, tag="biasO")
            for co in range(C):
                pq2 = pacc.tile([128, BL], F32, tag="acc")
                for ci in range(C):
                    nc.tensor.matmul(
                        pq2,
                        lhsT=wqv[:, ci, ts(co, 128)],
                        rhs=uT[:, ci, :],
                        start=(ci == 0),
                        stop=(ci == C - 1),
                    )
                nc.vector.tensor_scalar_add(biasO[:, co, :], pq2, qvb[:, co : co + 1])

            # ---------- options branch (tanh from staged kpds) ----------
            mdt = stream.tile([128, C, BL, NOPT, LO], BF16, tag="mdt")
            for co in range(C):
                for h in range(2):
                    for bq in range(2):
                        b = 2 * h + bq
                        nc.scalar.activation(
                            mdt[:, co, b],
                            kpds[:, co, h, ds(bq * NOPT * LO, NOPT * LO)],
                            AF.Tanh,
                            bias=biasO[:, co, b : b + 1],
                        )

            s_d = one.tile([1, BO * LO], F32R, tag="s_d")
            for h in range(2):
                lgd = prow.tile([1, HALF], F32, tag="lg")
                for co in range(C):
                    nc.tensor.matmul(
                        lgd,
                        lhsT=vwd[:, co : co + 1],
                        rhs=mdt[:, co, ds(2 * h, 2)],
                        start=(co == 0),
                        stop=(co == C - 1),
                    )
                nc.scalar.activation(s_d[:, ds(h * HALF, HALF)], lgd, AF.Exp)

            sums_d = one.tile([1, BO], F32, tag="sums_d")
            nc.vector.tensor_reduce(
                sums_d,
                s_d.bitcast(F32).rearrange("p (bo l) -> p bo l", l=LO),
                axis=AX.X,
                op=ALU.add,
            )
            rec_d = one.tile([1, BO], F32, tag="rec_d")
            nc.vector.reciprocal(rec_d, sums_d)
            prec = psml.tile([128, BO], F32, tag="sml")
            nc.tensor.matmul(
                prec, lhsT=ones.bitcast(F32), rhs=rec_d, start=True, stop=True
            )
            rec_rep = one.tile([128, BO], F32, tag="rec_rep")
            nc.scalar.copy(rec_rep, prec)

            # replicate exp scores with 1/sum folded in: sdn = s_d * rec
            sdn = rdpool.tile([128, BO, LO], BF16, tag="sdn")
            for h in range(2):
                prepd = pprep.tile([128, HALF], F32, tag="prep")
                nc.tensor.matmul(
                    prepd,
                    lhsT=ones,
                    rhs=s_d[:, ds(h * HALF, HALF)],
                    start=True,
                    stop=True,
                )
                nc.vector.scalar_tensor_tensor(
                    sdn[:, ds(h * BOH, BOH)],
                    rec_rep[:, ds(h * BOH, BOH)]
                    .unsqueeze(2)
                    .broadcast_to((128, BOH, LO)),
                    1.0,
                    prepd.rearrange("p (bo l) -> p bo l", l=LO),
                    op0=ALU.mult,
                    op1=ALU.mult,
                )

            # final linear on the PE over the score-scaled option tile:
            # logitsT[j, (b,l)] += fw[:,o,c,:].T @ (OT[:,c] * sdn)[:, :, o, :]
            OTf = OT.rearrange("p c b o l -> p c (b o) l")
            pout = psml.tile([OUTP, BL * LO], F32, tag="sml")
            for c in range(C):
                scrd = scratch.tile([128, BO, LO], BF16, tag="scrd", bufs=2)
                nc.vector.tensor_mul(scrd, OTf[:, c], sdn)
                sv = scrd.rearrange("p (b o) l -> p b o l", o=NOPT)
                for o in range(NOPT):
                    nc.tensor.matmul(
                        pout,
                        lhsT=fw[:, o, c, :],
                        rhs=sv[:, :, o, :],
                        start=(c == 0 and o == 0),
                        stop=(c == C - 1 and o == NOPT - 1),
                    )
            # reduce over l, add bias, store logits^T (host un-transposes)
            outsum = one.tile([OUTP, BL], F32, tag="outsum")
            nc.vector.tensor_reduce(
                outsum,
                pout.rearrange("p (b l) -> p b l", l=LO),
                axis=AX.X,
                op=ALU.add,
            )
            out_s = one.tile([OUTP, BL], F32, tag="out_s")
            nc.vector.tensor_scalar_add(out_s, outsum, fb)
            nc.sync.dma_start(out=outd, in_=out_s[0:OUT, :])

    nc.compile()
    return nc


@functools.lru_cache(maxsize=1)
def get_nc() -> bass.Bass:
    return build_nc()


def _swz(mat: np.ndarray) -> np.ndarray:
    """[H_in, X] -> [128, C, X]: partition-contiguous chunk swizzle."""
    return np.ascontiguousarray(
        mat.reshape(C, 128, -1).transpose(1, 0, 2)
    )


def make_in_maps(inputs: dict) -> list[dict]:
    art = np.ascontiguousarray(np.asarray(inputs["article_contexts"], np.float32))
    qc = np.ascontiguousarray(np.asarray(inputs["question_contexts"], np.float32))
    opt = np.ascontiguousarray(np.asarray(inputs["options_embeds"], np.float32))
    idx = np.asarray(inputs["answer_indices"]).astype(np.int64)

    def g(name):
        return np.asarray(inputs[name], np.float32)

    aQwT = _swz(np.ascontiguousarray(g("a_Qw").T))  # [128, C, H]
    aKwT = _swz(np.ascontiguousarray(g("a_Kw").T))
    wqk = np.stack([aQwT, aKwT], axis=1).astype(NP_F8)  # [128, 2, C, H]
    dKwT = _swz(np.ascontiguousarray(g("d_Kw").T)).astype(NP_BF16)
    # folded: aq -> options query projection
    Wqv = g("d_Qw") @ g("a_Vw")  # [H, H] (a_Vw maps h_in->h_out as aq = u @ a_Vw^T)
    qvwT = _swz(np.ascontiguousarray(Wqv.T.astype(np.float32))).astype(NP_BF16)
    bias_qv = g("d_Qw") @ g("a_Vb") + g("d_Qb") + g("d_Kb")  # [H]
    # folded: per-option final weights
    # feats[b,o,:] = u_d[b,o] @ d_Vw^T + d_Vb ; logits = sum_o feats[b,o] @ f_w[:,o]^T + f_b
    # => logits = sum_o u_d[b,o] @ (d_Vw^T @ f_w[:,o]^T) + (f_b + sum_o f_w[:,o] @ d_Vb)
    f_w = g("f_w")  # [OUT, 5H], flattened o-major
    dVwT = g("d_Vw").T  # [H_in, H_out]
    Ff = np.stack(
        [dVwT @ f_w[:, o * H : (o + 1) * H].T for o in range(NOPT)], axis=0
    )  # [o, H_in, OUT]
    fb_new = g("f_b") + sum(
        f_w[:, o * H : (o + 1) * H] @ g("d_Vb") for o in range(NOPT)
    )  # [OUT]
    fwT = np.zeros((128, NOPT, C, 8), np.float32)
    fwT[:, :, :, :OUT] = Ff.reshape(NOPT, C, 128, OUT).transpose(2, 0, 1, 3)
    fwT = fwT.astype(NP_BF16)

    def colvec(v):  # [H] -> [128, C] chunk-major
        return np.ascontiguousarray(np.asarray(v, np.float32).reshape(C, 128).T)

    vwa_col = colvec(g("a_vw").reshape(H))  # [128, C]
    vwaT = np.zeros((128, C2, 2, 16), np.float32)
    vwaT[:, :, :, 0] = vwa_col.reshape(128, C2, 2)
    vwaT = vwaT.astype(NP_F8)
    vwdT = colvec(g("d_vw").reshape(H)).astype(NP_BF16)
    qkbT = colvec(g("a_Qb") + g("a_Kb"))
    qvbT = colvec(bias_qv)

    # [B, H, LA] -> partition-swizzled [B, 128, C, LA]
    artT = (
        art.transpose(0, 2, 1)
        .reshape(B, C, 128, LA)
        .transpose(0, 2, 1, 3)
    )
    artT = np.ascontiguousarray(artT).astype(NP_F8)
    # [B, H, 5, LO] -> [B, 128, C, 5, LO]
    optT = (
        opt.transpose(0, 3, 1, 2)
        .reshape(B, C, 128, NOPT, LO)
        .transpose(0, 2, 1, 3, 4)
    )
    optT = np.ascontiguousarray(optT).astype(NP_BF16)
    onehot = np.zeros((B, LQ), np.float32)
    onehot[np.arange(B), idx] = 1.0
    onehot = onehot.astype(NP_F8)

    shared = dict(
        wqk=wqk, qvwT=qvwT, dKwT=dKwT,
        vwaT=vwaT, vwdT=vwdT, qkbT=qkbT, qvbT=qvbT,
        fwT=fwT,
        fb=np.ascontiguousarray(
            np.pad(fb_new.astype(np.float32), (0, 3)).reshape(OUTP, 1)
        ),
        ones1=np.ones((1, 128), np.float32),
    )
    qc8 = qc.astype(NP_F8)
    in_maps = []
    for r in range(NCORES):
        s = slice(r * BL, (r + 1) * BL)
        m = dict(shared)
        m["artT"] = artT[s]
        m["optT"] = optT[s]
        m["qc"] = qc8[s]
        m["oh"] = np.ascontiguousarray(onehot[s].T)
        in_maps.append(m)
    return in_maps


def run(inputs: dict, trace: bool = False, tmpdir=None):
    from concourse.bass_utils import run_bass_kernel_spmd

    nc = get_nc()
    in_maps = make_in_maps(inputs)
    res = run_bass_kernel_spmd(
        nc, in_maps, core_ids=list(range(NCORES)), trace=trace, tmpdir=tmpdir
    )
    out = np.concatenate(
        [res.results[r]["out"].T for r in range(NCORES)], axis=0
    )
    return out, res


def kernel(**inputs) -> np.ndarray:
    out, _ = run(inputs, trace=False)
    return out


# revision 18
# speedup vs baseline: 1.1221x; 1.1221x over previous
"""Trainium2 Bass kernel for nn_BaselineOut (article/option additive-attention MRC head).

Contract: kernel(**inputs) takes FULL unsharded inputs (numpy), returns FULL
[32, 5] float32 logits.  Internally: data-parallel over batch across 8 cores
(4 batch items per core), all params replicated.

Math notes (vs reference):
  - oqc gather is done as a one-hot matmul on device (host only encodes the
    int indices as a one-hot matrix - a layout/encoding transform).
  - V-projection is pulled out of the attention sum by linearity:
        sum_l softmax_l * (V @ Vw^T + Vb) = (sum_l softmax_l * V) @ Vw^T + Vb
    so the [B*L,H]x[H,H] V matmul collapses to a weighted sum over L plus a
    tiny [B,H]x[H,H] matmul.
  - Consecutive linear maps with no nonlinearity between are constant-folded
    on host (weight-weight products):
      * aq -> Qp_d: one matmul with Wqv = d_Qw @ a_Vw^T and a folded bias.
      * feats -> logits: per-option folded weights Ff_o = d_Vw^T @ f_w[:,o]^T.
  - softmax logit bias (vb) is dropped: softmax is shift-invariant.
  - exp is computed without max-subtraction: |logit| <= ||vw||_1 ~ 36, well
    inside fp32 exp range.
  - The article branch (Q/K projections, tanh dot, weighted sum) runs in
    fp8e4 with DoubleRow matmuls (2 k-chunks per instruction).  This is safe:
    the article attention output u only reaches the logits as a small
    additive query shift (std ~0.07) inside the option tanh (argument std
    ~1.4), so multi-percent noise in u damps to ~0.3% at the output.  The
    option branch K-projection stays bf16-or-better since its noise
    propagates ~1:1 into the logits.
  - The weighted V-sum is one fused DVE pass per h-chunk:
    scalar_tensor_tensor(T * scores_psum) with accum_out.
  - The option-branch normalization 1/sum is folded into the replicated
    score tile (sdn), and the final linear runs on the PE over the
    score-scaled option tile, producing logits^T; the host un-transposes.
  - Big tensors are host-swizzled so each SBUF partition's data is one long
    contiguous DRAM run (SWDGE throughput ~ run_length; wq/wk are fused into
    one 16KB-run tensor).  fp8/bf16 casting happens on host.  f32r must
    never be a DRAM I/O dtype (crashes NRT) - the one f32->f32r cast (ones)
    rides SWDGE.
  - The option K-projection (kpd) matmuls are issued before the uT/biasO
    chain and staged to SBUF via vector copies, so the PE fills the
    article->options dependency gap instead of idling.
"""

import functools
import sys

import numpy as np

sys.path.insert(0, "/opt/trn_rl_repo")

import ml_dtypes  # noqa: E402

import concourse.bass as bass  # noqa: E402
from concourse import bacc  # noqa: E402
import concourse.tile as tile  # noqa: E402
from concourse import mybir  # noqa: E402
from concourse.bass import ds, ts  # noqa: E402

B, LA, LQ, LO, H, OUT = 32, 2048, 64, 32, 1024, 5
NCORES = 8
BL = B // NCORES  # 4 batch items per core
NOPT = 5
F32 = mybir.dt.float32
F32R = mybir.dt.float32r
F8 = mybir.dt.float8e4
BF16 = mybir.dt.bfloat16
LT = 512  # article l-tile (free dim of the big matmuls)
NLT = LA // LT  # 4
C = H // 128  # 8 h-chunks
C2 = C // 2  # 4 h-chunk pairs (DoubleRow)
BO = BL * NOPT  # 20 (b, option) pairs per core
AF = mybir.ActivationFunctionType
ALU = mybir.AluOpType
AX = mybir.AxisListType
DR = mybir.MatmulPerfMode.DoubleRow
OUTP = 8  # final-linear out dim padded even
HALF = 2 * NOPT * LO  # 320 option columns (2 batch items)
BOH = BO // 2  # 10 (b,o) pairs per half
NP_F8 = ml_dtypes.float8_e4m3
NP_BF16 = ml_dtypes.bfloat16


def build_nc() -> bass.Bass:
    nc = bacc.Bacc("TRN2", target_bir_lowering=False, debug=False)

    # ---- DRAM I/O (per-core shard; names are the in_map keys) ----
    # Big tensors are pre-swizzled on host to [128(partition), ...contiguous].
    artT = nc.dram_tensor("artT", [BL, 128, C, LA], F8, kind="ExternalInput").ap()
    optT = nc.dram_tensor(
        "optT", [BL, 128, C, NOPT, LO], BF16, kind="ExternalInput"
    ).ap()
    qcd = nc.dram_tensor("qc", [BL, LQ, H], F8, kind="ExternalInput").ap()
    ohd = nc.dram_tensor("oh", [LQ, BL], F8, kind="ExternalInput").ap()
    wqkd = nc.dram_tensor("wqk", [128, 2, C, H], F8, kind="ExternalInput").ap()
    wQV = nc.dram_tensor("qvwT", [128, C, H], BF16, kind="ExternalInput").ap()
    wKd = nc.dram_tensor("dKwT", [128, C, H], BF16, kind="ExternalInput").ap()
    vwad = nc.dram_tensor("vwaT", [128, C2, 2, 16], F8, kind="ExternalInput").ap()
    vwdd = nc.dram_tensor("vwdT", [128, C], BF16, kind="ExternalInput").ap()
    qkbd = nc.dram_tensor("qkbT", [128, C], F32, kind="ExternalInput").ap()
    qvbd = nc.dram_tensor("qvbT", [128, C], F32, kind="ExternalInput").ap()
    fwd = nc.dram_tensor("fwT", [128, NOPT, C, OUTP], BF16, kind="ExternalInput").ap()
    fbd = nc.dram_tensor("fb", [OUTP, 1], F32, kind="ExternalInput").ap()
    onesd = nc.dram_tensor("ones1", [1, 128], F32, kind="ExternalInput").ap()
    outd = nc.dram_tensor("out", [OUT, BL], F32, kind="ExternalOutput").ap()

    with (
        tile.TileContext(nc) as tc,
        nc.allow_low_precision(reason="fp8/bf16 article branch; PE accums fp32"),
    ):
        with (
            tc.tile_pool(name="stream", bufs=1) as stream,
            tc.tile_pool(name="art", bufs=2) as artp,
            tc.tile_pool(name="wbig", bufs=1) as wbig,
            tc.tile_pool(name="mpool", bufs=3) as mpool,
            tc.tile_pool(name="spool", bufs=2) as spool,
            tc.tile_pool(name="rdpool", bufs=1) as rdpool,
            tc.tile_pool(name="ubuf", bufs=2) as ubuf,
            tc.tile_pool(name="scratch", bufs=1) as scratch,
            tc.tile_pool(name="one", bufs=1) as one,
            tc.tile_pool(name="pacc", bufs=3, space="PSUM") as pacc,
            tc.tile_pool(name="pprep", bufs=2, space="PSUM") as pprep,
            tc.tile_pool(name="prow", bufs=2, space="PSUM") as prow,
            tc.tile_pool(name="psml", bufs=1, space="PSUM") as psml,
        ):
            # ---------- SWDGE (gpsimd): the latency-critical byte loads ----
            # Order: head tensors (oqc/biasA inputs) first so the PE head
            # overlaps the big article loads.
            qct = stream.tile([LQ, BL, H], F8, tag="qct")
            for b in range(BL):
                nc.gpsimd.dma_start(out=qct[:, b, :], in_=qcd[b])
            oht = one.tile([LQ, BL], F8, tag="oht")
            nc.gpsimd.dma_start(out=oht, in_=ohd)
            wqk = wbig.tile([128, 2, C, H], F8, tag="wqk")
            nc.gpsimd.dma_start(out=wqk, in_=wqkd)
            wq = wqk[:, 0]
            wk = wqk[:, 1]
            art = [artp.tile([128, C, LA], F8, tag="art", name=f"art{b}")
                   for b in range(BL)]
            nc.gpsimd.dma_start(out=art[0], in_=artT[0])
            ones = one.tile([1, 128], F32R, tag="ones")
            nc.gpsimd.dma_start(out=ones, in_=onesd)
            nc.gpsimd.dma_start(out=art[1], in_=artT[1])

            # ---------- HWDGE (sync): consts + option-branch weights ----------
            # v-dot weights, padded to a 16B k-pair stride (dual-fp8
            # ldweights ISA restriction); column 0 is the real vw value.
            vwa = one.tile([128, C2, 2, 16], F8, tag="vwa")
            nc.sync.dma_start(out=vwa, in_=vwad)
            vwd = one.tile([128, C], BF16, tag="vwd")
            nc.sync.dma_start(out=vwd, in_=vwdd)
            qkb = one.tile([128, C], F32, tag="qkb")
            nc.sync.dma_start(out=qkb, in_=qkbd)
            qvb = one.tile([128, C], F32, tag="qvb")
            nc.sync.dma_start(out=qvb, in_=qvbd)
            fw = one.tile([128, NOPT, C, OUTP], BF16, tag="fw")
            nc.sync.dma_start(out=fw, in_=fwd)
            fb = one.tile([OUTP, 1], F32, tag="fb")
            nc.sync.dma_start(out=fb, in_=fbd)
            wqv = wbig.tile([128, C, H], BF16, tag="w", bufs=2)
            wdk = wbig.tile([128, C, H], BF16, tag="w", bufs=2)
            nc.sync.dma_start(out=wqv, in_=wQV)
            nc.sync.dma_start(out=wdk, in_=wKd)
            OT = stream.tile([128, C, BL, NOPT, LO], BF16, tag="ot")
            for b in range(BL):
                nc.sync.dma_start(out=OT[:, :, b], in_=optT[b])

            # ---------- gather oqc via one-hot matmul ----------
            oqcT = one.tile([128, C, BL], F8, tag="oqcT")
            for c in range(C):
                po = pacc.tile([128, BL], F32, tag="acc")
                for b in range(BL):
                    nc.tensor.matmul(
                        po[:, b : b + 1],
                        lhsT=qct[:, b, ts(c, 128)],
                        rhs=oht[:, b : b + 1],
                        start=True,
                        stop=True,
                    )
                nc.vector.tensor_copy(oqcT[:, c, :], po)

            # ---------- Qp^T = aQw @ oqc^T ; article tanh bias ----------
            biasA = one.tile([128, C, BL], F32, tag="biasA")
            for co in range(C):
                pq = pacc.tile([128, BL], F32, tag="acc")
                for ci in range(C):
                    nc.tensor.matmul(
                        pq,
                        lhsT=wq[:, ci, ts(co, 128)],
                        rhs=oqcT[:, ci, :],
                        start=(ci == 0),
                        stop=(ci == C - 1),
                    )
                nc.vector.tensor_scalar_add(biasA[:, co, :], pq, qkb[:, co : co + 1])

            # ---------- article branch ----------
            # Per (b, lt): fp8 DoubleRow K-projection -> tanh (+bias) -> fp8
            # DoubleRow v-dot -> exp -> PE-replicated scores -> fused
            # multiply+reduce weighted V-sum on DVE.
            s_sums = one.tile([1, BL, NLT], F32, tag="s_sums")
            uTun = one.tile([128, C, BL], F32, tag="uTun")
            for b in range(BL):
                if b + 2 <= BL - 1:
                    nc.gpsimd.dma_start(out=art[b + 2], in_=artT[b + 2])
                T = art[b]
                upart = ubuf.tile([128, C, NLT], F32, tag="upart")
                for lt in range(NLT):
                    lg = prow.tile([2, LT], F32, tag="lg")
                    for cop in range(C2):
                        mt2 = mpool.tile([128, 2, LT], F8, tag="mt")
                        for half in range(2):
                            co = 2 * cop + half
                            kp = pacc.tile([128, LT], F32, tag="acc")
                            for ci2 in range(C2):
                                nc.tensor.matmul(
                                    kp,
                                    lhsT=wk[:, 2 * ci2 : 2 * ci2 + 2, ts(co, 128)],
                                    rhs=T[:, 2 * ci2 : 2 * ci2 + 2, ds(lt * LT, LT)],
                                    start=(ci2 == 0),
                                    stop=(ci2 == C2 - 1),
                                    perf_mode=DR,
                                )
                            nc.scalar.activation(
                                mt2[:, half, :], kp, AF.Tanh,
                                bias=biasA[:, co, b : b + 1],
                            )
                        nc.tensor.matmul(
                            lg,
                            lhsT=vwa[:, cop, :, 0:2],
                            rhs=mt2,
                            start=(cop == 0),
                            stop=(cop == C2 - 1),
                            perf_mode=DR,
                        )
                    st = spool.tile([1, LT], F32R, tag="st")
                    nc.scalar.activation(
                        st, lg[0:1, :], AF.Exp, accum_out=s_sums[:, b, lt : lt + 1]
                    )
                    # replicate s~ across partitions: ones^T (x) st via PE
                    prep = pprep.tile([128, LT], F32, tag="prep")
                    nc.tensor.matmul(prep, lhsT=ones, rhs=st, start=True, stop=True)
                    # fused weighted V-sum: upart[:,c,lt] = sum_l T*s, one pass
                    for c in range(C):
                        scr = scratch.tile([128, LT], F32, tag="scr")
                        nc.vector.scalar_tensor_tensor(
                            scr,
                            T[:, c, ds(lt * LT, LT)],
                            1.0,
                            prep,
                            op0=ALU.mult,
                            op1=ALU.mult,
                            accum_out=upart[:, c, lt : lt + 1],
                        )
                # sum the NLT partial weighted sums -> unnormalized u^T
                nc.vector.tensor_reduce(
                    uTun[:, :, b : b + 1], upart, axis=AX.X, op=ALU.add
                )

            # ---------- options K-projection, issued before the biasO chain
            # so the PE fills the article->options dependency gap; results
            # staged to SBUF via vector copies (PSUM ring stays small).
            kpds = stream.tile([128, C, 2, HALF], F32, tag="kpds")
            for co in range(C):
                for h in range(2):
                    kpd = pacc.tile([128, HALF], F32, tag="acc")
                    for ci in range(C):
                        nc.tensor.matmul(
                            kpd,
                            lhsT=wdk[:, ci, ts(co, 128)],
                            rhs=OT[:, ci, ds(2 * h, 2)],
                            start=(ci == 0),
                            stop=(ci == C - 1),
                        )
                    nc.vector.tensor_copy(kpds[:, co, h], kpd)

            # normalization factors: 1/sum(exp) per b, replicated to 128 parts
            ssb = one.tile([1, BL], F32, tag="ssb")
            nc.vector.tensor_reduce(ssb, s_sums, axis=AX.X, op=ALU.add)
            psb = psml.tile([128, BL], F32, tag="sml")
            nc.tensor.matmul(
                psb, lhsT=ones.bitcast(F32), rhs=ssb, start=True, stop=True
            )
            rs_rep = one.tile([128, BL], F32, tag="rs_rep")
            nc.vector.reciprocal(rs_rep, psb)

            uT = one.tile([128, C, BL], BF16, tag="uT")
            for b in range(BL):
                nc.vector.tensor_scalar_mul(
                    uT[:, :, b], uTun[:, :, b], rs_rep[:, b : b + 1]
                )

            # ---------- option tanh bias via folded Wqv = d_Qw a_Vw^T ----------
            biasO = one.tile([128, C, BL], F32, tag="biasO")
            for co in range(C):
                pq2 = pacc.tile([128, BL], F32, tag="acc")
                for ci in range(C):
                    nc.tensor.matmul(
                        pq2,
                        lhsT=wqv[:, ci, ts(co, 128)],
                        rhs=uT[:, ci, :],
                        start=(ci == 0),
                        stop=(ci == C - 1),
                    )
                nc.vector.tensor_scalar_add(biasO[:, co, :], pq2, qvb[:, co : co + 1])

            # ---------- options branch (tanh from staged kpds) ----------
            mdt = stream.tile([128, C, BL, NOPT, LO], BF16, tag="mdt")
            for co in range(C):
                for h in range(2):
                    for bq in range(2):
                        b = 2 * h + bq
                        nc.scalar.activation(
                            mdt[:, co, b],
                            kpds[:, co, h, ds(bq * NOPT * LO, NOPT * LO)],
                            AF.Tanh,
                            bias=biasO[:, co, b : b + 1],
                        )

            s_d = one.tile([1, BO * LO], F32R, tag="s_d")
            for h in range(2):
                lgd = prow.tile([1, HALF], F32, tag="lg")
                for co in range(C):
                    nc.tensor.matmul(
                        lgd,
                        lhsT=vwd[:, co : co + 1],
                        rhs=mdt[:, co, ds(2 * h, 2)],
                        start=(co == 0),
                        stop=(co == C - 1),
                    )
                nc.scalar.activation(s_d[:, ds(h * HALF, HALF)], lgd, AF.Exp)

            sums_d = one.tile([1, BO], F32, tag="sums_d")
            nc.vector.tensor_reduce(
                sums_d,
                s_d.bitcast(F32).rearrange("p (bo l) -> p bo l", l=LO),
                axis=AX.X,
                op=ALU.add,
            )
            rec_d = one.tile([1, BO], F32, tag="rec_d")
            nc.vector.reciprocal(rec_d, sums_d)
            prec = psml.tile([128, BO], F32, tag="sml")
            nc.tensor.matmul(
                prec, lhsT=ones.bitcast(F32), rhs=rec_d, start=True, stop=True
            )
            rec_rep = one.tile([128, BO], F32, tag="rec_rep")
            nc.scalar.copy(rec_rep, prec)

            # replicate exp scores with 1/sum folded in: sdn = s_d * rec
            sdn = rdpool.tile([128, BO, LO], BF16, tag="sdn")
            for h in range(2):
                prepd = pprep.tile([128, HALF], F32, tag="prep")
                nc.tensor.matmul(
                    prepd,
                    lhsT=ones,
                    rhs=s_d[:, ds(h * HALF, HALF)],
                    start=True,
                    stop=True,
                )
                nc.vector.scalar_tensor_tensor(
                    sdn[:, ds(h * BOH, BOH)],
                    rec_rep[:, ds(h * BOH, BOH)]
                    .unsqueeze(2)
                    .broadcast_to((128, BOH, LO)),
                    1.0,
                    prepd.rearrange("p (bo l) -> p bo l", l=LO),
                    op0=ALU.mult,
                    op1=ALU.mult,
                )

            # final linear on the PE over the score-scaled option tile:
            # logitsT[j, (b,l)] += fw[:,o,c,:].T @ (OT[:,c] * sdn)[:, :, o, :]
            OTf = OT.rearrange("p c b o l -> p c (b o) l")
            pout = psml.tile([OUTP, BL * LO], F32, tag="sml")
            for c in range(C):
                scrd = scratch.tile([128, BO, LO], BF16, tag="scrd", bufs=2)
                nc.vector.tensor_mul(scrd, OTf[:, c], sdn)
                sv = scrd.rearrange("p (b o) l -> p b o l", o=NOPT)
                for o in range(NOPT):
                    nc.tensor.matmul(
                        pout,
                        lhsT=fw[:, o, c, :],
                        rhs=sv[:, :, o, :],
                        start=(c == 0 and o == 0),
                        stop=(c == C - 1 and o == NOPT - 1),
                    )
            # reduce over l, add bias, store logits^T (host un-transposes)
            outsum = one.tile([OUTP, BL], F32, tag="outsum")
            nc.vector.tensor_reduce(
                outsum,
                pout.rearrange("p (b l) -> p b l", l=LO),
                axis=AX.X,
                op=ALU.add,
            )
            out_s = one.tile([OUTP, BL], F32, tag="out_s")
            nc.vector.tensor_scalar_add(out_s, outsum, fb)
            nc.sync.dma_start(out=outd, in_=out_s[0:OUT, :])

    nc.compile()
    return nc


@functools.lru_cache(maxsize=1)
def get_nc() -> bass.Bass:
    return build_nc()


def _swz(mat: np.ndarray) -> np.ndarray:
    """[H_in, X] -> [128, C, X]: partition-contiguous chunk swizzle."""
    return np.ascontiguousarray(
        mat.reshape(C, 128, -1).transpose(1, 0, 2)
    )


def make_in_maps(inputs: dict) -> list[dict]:
    art = np.ascontiguousarray(np.asarray(inputs["article_contexts"], np.float32))
    qc = np.ascontiguousarray(np.asarray(inputs["question_contexts"], np.float32))
    opt = np.ascontiguousarray(np.asarray(inputs["options_embeds"], np.float32))
    idx = np.asarray(inputs["answer_indices"]).astype(np.int64)

    def g(name):
        return np.asarray(inputs[name], np.float32)

    aQwT = _swz(np.ascontiguousarray(g("a_Qw").T))  # [128, C, H]
    aKwT = _swz(np.ascontiguousarray(g("a_Kw").T))
    wqk = np.stack([aQwT, aKwT], axis=1).astype(NP_F8)  # [128, 2, C, H]
    dKwT = _swz(np.ascontiguousarray(g("d_Kw").T)).astype(NP_BF16)
    # folded: aq -> options query projection
    Wqv = g("d_Qw") @ g("a_Vw")  # [H, H] (a_Vw maps h_in->h_out as aq = u @ a_Vw^T)
    qvwT = _swz(np.ascontiguousarray(Wqv.T.astype(np.float32))).astype(NP_BF16)
    bias_qv = g("d_Qw") @ g("a_Vb") + g("d_Qb") + g("d_Kb")  # [H]
    # folded: per-option final weights
    # feats[b,o,:] = u_d[b,o] @ d_Vw^T + d_Vb ; logits = sum_o feats[b,o] @ f_w[:,o]^T + f_b
    # => logits = sum_o u_d[b,o] @ (d_Vw^T @ f_w[:,o]^T) + (f_b + sum_o f_w[:,o] @ d_Vb)
    f_w = g("f_w")  # [OUT, 5H], flattened o-major
    dVwT = g("d_Vw").T  # [H_in, H_out]
    Ff = np.stack(
        [dVwT @ f_w[:, o * H : (o + 1) * H].T for o in range(NOPT)], axis=0
    )  # [o, H_in, OUT]
    fb_new = g("f_b") + sum(
        f_w[:, o * H : (o + 1) * H] @ g("d_Vb") for o in range(NOPT)
    )  # [OUT]
    fwT = np.zeros((128, NOPT, C, 8), np.float32)
    fwT[:, :, :, :OUT] = Ff.reshape(NOPT, C, 128, OUT).transpose(2, 0, 1, 3)
    fwT = fwT.astype(NP_BF16)

    def colvec(v):  # [H] -> [128, C] chunk-major
        return np.ascontiguousarray(np.asarray(v, np.float32).reshape(C, 128).T)

    vwa_col = colvec(g("a_vw").reshape(H))  # [128, C]
    vwaT = np.zeros((128, C2, 2, 16), np.float32)
    vwaT[:, :, :, 0] = vwa_col.reshape(128, C2, 2)
    vwaT = vwaT.astype(NP_F8)
    vwdT = colvec(g("d_vw").reshape(H)).astype(NP_BF16)
    qkbT = colvec(g("a_Qb") + g("a_Kb"))
    qvbT = colvec(bias_qv)

    # [B, H, LA] -> partition-swizzled [B, 128, C, LA]
    artT = (
        art.transpose(0, 2, 1)
        .reshape(B, C, 128, LA)
        .transpose(0, 2, 1, 3)
    )
    artT = np.ascontiguousarray(artT).astype(NP_F8)
    # [B, H, 5, LO] -> [B, 128, C, 5, LO]
    optT = (
        opt.transpose(0, 3, 1, 2)
        .reshape(B, C, 128, NOPT, LO)
        .transpose(0, 2, 1, 3, 4)
    )
    optT = np.ascontiguousarray(optT).astype(NP_BF16)
    onehot = np.zeros((B, LQ), np.float32)
    onehot[np.arange(B), idx] = 1.0
    onehot = onehot.astype(NP_F8)

    shared = dict(
        wqk=wqk, qvwT=qvwT, dKwT=dKwT,
        vwaT=vwaT, vwdT=vwdT, qkbT=qkbT, qvbT=qvbT,
        fwT=fwT,
        fb=np.ascontiguousarray(
            np.pad(fb_new.astype(np.float32), (0, 3)).reshape(OUTP, 1)
        ),
        ones1=np.ones((1, 128), np.float32),
    )
    qc8 = qc.astype(NP_F8)
    in_maps = []
    for r in range(NCORES):
        s = slice(r * BL, (r + 1) * BL)
        m = dict(shared)
        m["artT"] = artT[s]
        m["optT"] = optT[s]
        m["qc"] = qc8[s]
        m["oh"] = np.ascontiguousarray(onehot[s].T)
        in_maps.append(m)
    return in_maps


def run(inputs: dict, trace: bool = False, tmpdir=None):
    from concourse.bass_utils import run_bass_kernel_spmd

    nc = get_nc()
    in_maps = make_in_maps(inputs)
    res = run_bass_kernel_spmd(
        nc, in_maps, core_ids=list(range(NCORES)), trace=trace, tmpdir=tmpdir
    )
    out = np.concatenate(
        [res.results[r]["out"].T for r in range(NCORES)], axis=0
    )
    return out, res


def kernel(**inputs) -> np.ndarray:
    out, _ = run(inputs, trace=False)
    return out


# revision 19
# speedup vs baseline: 1.1594x; 1.0332x over previous
"""Trainium2 Bass kernel for nn_BaselineOut (article/option additive-attention MRC head).

Contract: kernel(**inputs) takes FULL unsharded inputs (numpy), returns FULL
[32, 5] float32 logits.  Internally: data-parallel over batch across 8 cores
(4 batch items per core), all params replicated.

Math notes (vs reference):
  - oqc gather is done as a one-hot matmul on device (host only encodes the
    int indices as a one-hot matrix - a layout/encoding transform).
  - V-projection is pulled out of the attention sum by linearity:
        sum_l softmax_l * (V @ Vw^T + Vb) = (sum_l softmax_l * V) @ Vw^T + Vb
    so the [B*L,H]x[H,H] V matmul collapses to a weighted sum over L plus a
    tiny [B,H]x[H,H] matmul.
  - Consecutive linear maps with no nonlinearity between are constant-folded
    on host (weight-weight products):
      * aq -> Qp_d: one matmul with Wqv = d_Qw @ a_Vw^T and a folded bias.
      * feats -> logits: per-option folded weights Ff_o = d_Vw^T @ f_w[:,o]^T.
  - softmax logit bias (vb) is dropped: softmax is shift-invariant.
  - exp is computed without max-subtraction: |logit| <= ||vw||_1 ~ 36, well
    inside fp32 exp range.
  - The article branch (Q/K projections, tanh dot, weighted sum) runs in
    fp8e4 with DoubleRow matmuls (2 k-chunks per instruction).  This is safe:
    the article attention output u only reaches the logits as a small
    additive query shift (std ~0.07) inside the option tanh (argument std
    ~1.4), so multi-percent noise in u damps to ~0.3% at the output.  The
    option branch K-projection stays bf16-or-better since its noise
    propagates ~1:1 into the logits.
  - The weighted V-sum is one fused DVE pass per h-chunk:
    scalar_tensor_tensor(T * scores_psum) with accum_out.
  - The option-branch normalization 1/sum is folded into the replicated
    score tile (sdn), and the final linear runs on the PE over the
    score-scaled option tile, producing logits^T; the host un-transposes.
  - Big tensors are host-swizzled so each SBUF partition's data is one long
    contiguous DRAM run (SWDGE throughput ~ run_length; wq/wk are fused into
    one 16KB-run tensor).  fp8/bf16 casting happens on host.  f32r must
    never be a DRAM I/O dtype (crashes NRT) - the one f32->f32r cast (ones)
    rides SWDGE.
  - The option K-projection (kpd) matmuls are issued before the uT/biasO
    chain and staged to SBUF via vector copies, so the PE fills the
    article->options dependency gap instead of idling.
"""

import functools
import sys

import numpy as np

sys.path.insert(0, "/opt/trn_rl_repo")

import ml_dtypes  # noqa: E402

import concourse.bass as bass  # noqa: E402
from concourse import bacc  # noqa: E402
import concourse.tile as tile  # noqa: E402
from concourse import mybir  # noqa: E402
from concourse.bass import ds, ts  # noqa: E402

B, LA, LQ, LO, H, OUT = 32, 2048, 64, 32, 1024, 5
NCORES = 8
BL = B // NCORES  # 4 batch items per core
NOPT = 5
F32 = mybir.dt.float32
F32R = mybir.dt.float32r
F8 = mybir.dt.float8e4
BF16 = mybir.dt.bfloat16
LT = 512  # article l-tile (free dim of the big matmuls)
NLT = LA // LT  # 4
C = H // 128  # 8 h-chunks
C2 = C // 2  # 4 h-chunk pairs (DoubleRow)
BO = BL * NOPT  # 20 (b, option) pairs per core
AF = mybir.ActivationFunctionType
ALU = mybir.AluOpType
AX = mybir.AxisListType
DR = mybir.MatmulPerfMode.DoubleRow
OUTP = 8  # final-linear out dim padded even
HALF = 2 * NOPT * LO  # 320 option columns (2 batch items)
BOH = BO // 2  # 10 (b,o) pairs per half
NP_F8 = ml_dtypes.float8_e4m3
NP_BF16 = ml_dtypes.bfloat16


def build_nc() -> bass.Bass:
    nc = bacc.Bacc("TRN2", target_bir_lowering=False, debug=False)

    # ---- DRAM I/O (per-core shard; names are the in_map keys) ----
    # Big tensors are pre-swizzled on host to [128(partition), ...contiguous].
    artT = nc.dram_tensor("artT", [BL, 128, C, LA], F8, kind="ExternalInput").ap()
    optT = nc.dram_tensor(
        "optT", [BL, 128, C, NOPT, LO], BF16, kind="ExternalInput"
    ).ap()
    qcd = nc.dram_tensor("qc", [BL, LQ, H], F8, kind="ExternalInput").ap()
    ohd = nc.dram_tensor("oh", [LQ, BL], F8, kind="ExternalInput").ap()
    wqkd = nc.dram_tensor("wqk", [128, 2, C, H], F8, kind="ExternalInput").ap()
    wQV = nc.dram_tensor("qvwT", [128, C, H], BF16, kind="ExternalInput").ap()
    wKd = nc.dram_tensor("dKwT", [128, C, H], BF16, kind="ExternalInput").ap()
    vwad = nc.dram_tensor("vwaT", [128, C2, 2, 16], F8, kind="ExternalInput").ap()
    vwdd = nc.dram_tensor("vwdT", [128, C], BF16, kind="ExternalInput").ap()
    qkbd = nc.dram_tensor("qkbT", [128, C], F32, kind="ExternalInput").ap()
    qvbd = nc.dram_tensor("qvbT", [128, C], F32, kind="ExternalInput").ap()
    fwd = nc.dram_tensor("fwT", [128, NOPT, C, OUTP], BF16, kind="ExternalInput").ap()
    fbd = nc.dram_tensor("fb", [OUTP, 1], F32, kind="ExternalInput").ap()
    onesd = nc.dram_tensor("ones1", [1, 128], F32, kind="ExternalInput").ap()
    outd = nc.dram_tensor("out", [OUT, BL], F32, kind="ExternalOutput").ap()

    with (
        tile.TileContext(nc) as tc,
        nc.allow_low_precision(reason="fp8/bf16 article branch; PE accums fp32"),
    ):
        with (
            tc.tile_pool(name="stream", bufs=1) as stream,
            tc.tile_pool(name="art", bufs=2) as artp,
            tc.tile_pool(name="wbig", bufs=1) as wbig,
            tc.tile_pool(name="mpool", bufs=3) as mpool,
            tc.tile_pool(name="spool", bufs=2) as spool,
            tc.tile_pool(name="rdpool", bufs=1) as rdpool,
            tc.tile_pool(name="ubuf", bufs=2) as ubuf,
            tc.tile_pool(name="scratch", bufs=1) as scratch,
            tc.tile_pool(name="one", bufs=1) as one,
            tc.tile_pool(name="pacc", bufs=3, space="PSUM") as pacc,
            tc.tile_pool(name="pprep", bufs=2, space="PSUM") as pprep,
            tc.tile_pool(name="prow", bufs=2, space="PSUM") as prow,
            tc.tile_pool(name="psml", bufs=1, space="PSUM") as psml,
        ):
            # ---------- SWDGE (gpsimd): the latency-critical byte loads ----
            # Order: head tensors (oqc/biasA inputs) first so the PE head
            # overlaps the big article loads.
            qct = stream.tile([LQ, BL, H], F8, tag="qct")
            for b in range(BL):
                nc.gpsimd.dma_start(out=qct[:, b, :], in_=qcd[b])
            oht = one.tile([LQ, BL], F8, tag="oht")
            nc.gpsimd.dma_start(out=oht, in_=ohd)
            wqk = wbig.tile([128, 2, C, H], F8, tag="wqk")
            nc.gpsimd.dma_start(out=wqk, in_=wqkd)
            wq = wqk[:, 0]
            wk = wqk[:, 1]
            art = [artp.tile([128, C, LA], F8, tag="art", name=f"art{b}")
                   for b in range(BL)]
            nc.gpsimd.dma_start(out=art[0], in_=artT[0])
            ones = one.tile([1, 128], F32R, tag="ones")
            nc.gpsimd.dma_start(out=ones, in_=onesd)
            nc.gpsimd.dma_start(out=art[1], in_=artT[1])

            # ---------- HWDGE (sync): consts + option-branch weights ----------
            # v-dot weights, padded to a 16B k-pair stride (dual-fp8
            # ldweights ISA restriction); column 0 is the real vw value.
            vwa = one.tile([128, C2, 2, 16], F8, tag="vwa")
            nc.sync.dma_start(out=vwa, in_=vwad)
            vwd = one.tile([128, C], BF16, tag="vwd")
            nc.sync.dma_start(out=vwd, in_=vwdd)
            qkb = one.tile([128, C], F32, tag="qkb")
            nc.sync.dma_start(out=qkb, in_=qkbd)
            qvb = one.tile([128, C], F32, tag="qvb")
            nc.sync.dma_start(out=qvb, in_=qvbd)
            fw = one.tile([128, NOPT, C, OUTP], BF16, tag="fw")
            nc.sync.dma_start(out=fw, in_=fwd)
            fb = one.tile([OUTP, 1], F32, tag="fb")
            nc.sync.dma_start(out=fb, in_=fbd)
            wqv = wbig.tile([128, C, H], BF16, tag="w", bufs=2)
            wdk = wbig.tile([128, C, H], BF16, tag="w", bufs=2)
            nc.sync.dma_start(out=wqv, in_=wQV)
            nc.sync.dma_start(out=wdk, in_=wKd)
            OT = stream.tile([128, C, BL, NOPT, LO], BF16, tag="ot")
            for b in range(BL):
                nc.sync.dma_start(out=OT[:, :, b], in_=optT[b])

            # ---------- gather oqc via one-hot matmul ----------
            oqcT = one.tile([128, C, BL], F8, tag="oqcT")
            for c in range(C):
                po = pacc.tile([128, BL], F32, tag="acc")
                for b in range(BL):
                    nc.tensor.matmul(
                        po[:, b : b + 1],
                        lhsT=qct[:, b, ts(c, 128)],
                        rhs=oht[:, b : b + 1],
                        start=True,
                        stop=True,
                    )
                nc.vector.tensor_copy(oqcT[:, c, :], po)

            # ---------- Qp^T = aQw @ oqc^T ; article tanh bias ----------
            biasA = one.tile([128, C, BL], F32, tag="biasA")
            for co in range(C):
                pq = pacc.tile([128, BL], F32, tag="acc")
                for ci in range(C):
                    nc.tensor.matmul(
                        pq,
                        lhsT=wq[:, ci, ts(co, 128)],
                        rhs=oqcT[:, ci, :],
                        start=(ci == 0),
                        stop=(ci == C - 1),
                    )
                nc.vector.tensor_scalar_add(biasA[:, co, :], pq, qkb[:, co : co + 1])

            # ---------- article branch ----------
            # Per (b, lt): fp8 DoubleRow K-projection -> tanh (+bias) -> fp8
            # DoubleRow v-dot -> exp -> PE-replicated scores -> fused
            # multiply+reduce weighted V-sum on DVE.
            s_sums = one.tile([1, BL, NLT], F32, tag="s_sums")
            uTun = one.tile([128, C, BL], F32, tag="uTun")
            for b in range(BL):
                if b + 2 <= BL - 1:
                    nc.gpsimd.dma_start(out=art[b + 2], in_=artT[b + 2])
                T = art[b]
                upart = ubuf.tile([128, C, NLT], F32, tag="upart")
                for lt in range(NLT):
                    lg = prow.tile([2, LT], F32, tag="lg")
                    for cop in range(C2):
                        mt2 = mpool.tile([128, 2, LT], F8, tag="mt")
                        for half in range(2):
                            co = 2 * cop + half
                            kp = pacc.tile([128, LT], F32, tag="acc")
                            for ci2 in range(C2):
                                nc.tensor.matmul(
                                    kp,
                                    lhsT=wk[:, 2 * ci2 : 2 * ci2 + 2, ts(co, 128)],
                                    rhs=T[:, 2 * ci2 : 2 * ci2 + 2, ds(lt * LT, LT)],
                                    start=(ci2 == 0),
                                    stop=(ci2 == C2 - 1),
                                    perf_mode=DR,
                                )
                            nc.scalar.activation(
                                mt2[:, half, :], kp, AF.Tanh,
                                bias=biasA[:, co, b : b + 1],
                            )
                        nc.tensor.matmul(
                            lg,
                            lhsT=vwa[:, cop, :, 0:2],
                            rhs=mt2,
                            start=(cop == 0),
                            stop=(cop == C2 - 1),
                            perf_mode=DR,
                        )
                    st = spool.tile([1, LT], F32R, tag="st")
                    nc.scalar.activation(
                        st, lg[0:1, :], AF.Exp, accum_out=s_sums[:, b, lt : lt + 1]
                    )
                    # replicate s~ across partitions: ones^T (x) st via PE
                    prep = pprep.tile([128, LT], F32, tag="prep")
                    nc.tensor.matmul(prep, lhsT=ones, rhs=st, start=True, stop=True)
                    # fused weighted V-sum: upart[:,c,lt] = sum_l T*s, one pass
                    for c in range(C):
                        scr = scratch.tile([128, LT], F32, tag="scr")
                        nc.vector.scalar_tensor_tensor(
                            scr,
                            T[:, c, ds(lt * LT, LT)],
                            1.0,
                            prep,
                            op0=ALU.mult,
                            op1=ALU.mult,
                            accum_out=upart[:, c, lt : lt + 1],
                        )
                # sum the NLT partial weighted sums -> unnormalized u^T
                nc.vector.tensor_reduce(
                    uTun[:, :, b : b + 1], upart, axis=AX.X, op=ALU.add
                )

            # ---------- options K-projection, issued before the biasO chain
            # so the PE fills the article->options dependency gap; results
            # staged to SBUF via vector copies (PSUM ring stays small).
            kpds = stream.tile([128, C, 2, HALF], F32, tag="kpds")
            for co in range(C):
                for h in range(2):
                    kpd = pacc.tile([128, HALF], F32, tag="acc")
                    for ci in range(C):
                        nc.tensor.matmul(
                            kpd,
                            lhsT=wdk[:, ci, ts(co, 128)],
                            rhs=OT[:, ci, ds(2 * h, 2)],
                            start=(ci == 0),
                            stop=(ci == C - 1),
                        )
                    nc.vector.tensor_copy(kpds[:, co, h], kpd)

            # normalization factors: 1/sum(exp) per b, replicated to 128 parts
            ssb = one.tile([1, BL], F32, tag="ssb")
            nc.vector.tensor_reduce(ssb, s_sums, axis=AX.X, op=ALU.add)
            psb = psml.tile([128, BL], F32, tag="sml")
            nc.tensor.matmul(
                psb, lhsT=ones.bitcast(F32), rhs=ssb, start=True, stop=True
            )
            rs_rep = one.tile([128, BL], F32, tag="rs_rep")
            nc.vector.reciprocal(rs_rep, psb)

            uT = one.tile([128, C, BL], BF16, tag="uT")
            for b in range(BL):
                nc.vector.tensor_scalar_mul(
                    uT[:, :, b], uTun[:, :, b], rs_rep[:, b : b + 1]
                )

            # ---------- option tanh bias via folded Wqv = d_Qw a_Vw^T ----------
            biasO = one.tile([128, C, BL], F32, tag="biasO")
            for co in range(C):
                pq2 = pacc.tile([128, BL], F32, tag="acc")
                for ci in range(C):
                    nc.tensor.matmul(
                        pq2,
                        lhsT=wqv[:, ci, ts(co, 128)],
                        rhs=uT[:, ci, :],
                        start=(ci == 0),
                        stop=(ci == C - 1),
                    )
                nc.vector.tensor_scalar_add(biasO[:, co, :], pq2, qvb[:, co : co + 1])

            # ---------- options branch (tanh from staged kpds) ----------
            # biasO is broadcast-added into kpds on the vector engine (it
            # varies per b, so it can't be an activation bias for a merged
            # tile), then one big tanh per co on scalar; vector runs one co
            # ahead of scalar.
            mdt = stream.tile([128, C, BL, NOPT, LO], BF16, tag="mdt")
            BX = NOPT * LO  # 160 columns per batch item
            for co in range(C):
                kv = kpds[:, co].rearrange("p h (b x) -> p h b x", b=2)
                bv = (
                    biasO[:, co, :]
                    .rearrange("p (h b) -> p h b", b=2)
                    .unsqueeze(3)
                    .broadcast_to((128, 2, 2, BX))
                )
                nc.vector.scalar_tensor_tensor(
                    kv, kv, 0.0, bv, op0=ALU.add, op1=ALU.add
                )
                nc.scalar.activation(
                    mdt[:, co].rearrange("p b o l -> p (b o l)"),
                    kpds[:, co].rearrange("p h x -> p (h x)"),
                    AF.Tanh,
                )

            s_d = one.tile([1, BO * LO], F32R, tag="s_d")
            for h in range(2):
                lgd = prow.tile([1, HALF], F32, tag="lg")
                for co in range(C):
                    nc.tensor.matmul(
                        lgd,
                        lhsT=vwd[:, co : co + 1],
                        rhs=mdt[:, co, ds(2 * h, 2)],
                        start=(co == 0),
                        stop=(co == C - 1),
                    )
                nc.scalar.activation(s_d[:, ds(h * HALF, HALF)], lgd, AF.Exp)

            sums_d = one.tile([1, BO], F32, tag="sums_d")
            nc.vector.tensor_reduce(
                sums_d,
                s_d.bitcast(F32).rearrange("p (bo l) -> p bo l", l=LO),
                axis=AX.X,
                op=ALU.add,
            )
            rec_d = one.tile([1, BO], F32, tag="rec_d")
            nc.vector.reciprocal(rec_d, sums_d)
            prec = psml.tile([128, BO], F32, tag="sml")
            nc.tensor.matmul(
                prec, lhsT=ones.bitcast(F32), rhs=rec_d, start=True, stop=True
            )
            rec_rep = one.tile([128, BO], F32, tag="rec_rep")
            nc.scalar.copy(rec_rep, prec)

            # replicate exp scores with 1/sum folded in: sdn = s_d * rec
            sdn = rdpool.tile([128, BO, LO], BF16, tag="sdn")
            for h in range(2):
                prepd = pprep.tile([128, HALF], F32, tag="prep")
                nc.tensor.matmul(
                    prepd,
                    lhsT=ones,
                    rhs=s_d[:, ds(h * HALF, HALF)],
                    start=True,
                    stop=True,
                )
                nc.vector.scalar_tensor_tensor(
                    sdn[:, ds(h * BOH, BOH)],
                    rec_rep[:, ds(h * BOH, BOH)]
                    .unsqueeze(2)
                    .broadcast_to((128, BOH, LO)),
                    1.0,
                    prepd.rearrange("p (bo l) -> p bo l", l=LO),
                    op0=ALU.mult,
                    op1=ALU.mult,
                )

            # final linear on the PE over the score-scaled option tile:
            # logitsT[j, (b,l)] += fw[:,o,c,:].T @ (OT[:,c] * sdn)[:, :, o, :]
            OTf = OT.rearrange("p c b o l -> p c (b o) l")
            pout = psml.tile([OUTP, BL * LO], F32, tag="sml")
            for c in range(C):
                scrd = scratch.tile([128, BO, LO], BF16, tag="scrd", bufs=2)
                nc.vector.tensor_mul(scrd, OTf[:, c], sdn)
                sv = scrd.rearrange("p (b o) l -> p b o l", o=NOPT)
                for o in range(NOPT):
                    nc.tensor.matmul(
                        pout,
                        lhsT=fw[:, o, c, :],
                        rhs=sv[:, :, o, :],
                        start=(c == 0 and o == 0),
                        stop=(c == C - 1 and o == NOPT - 1),
                    )
            # reduce over l, add bias, store logits^T (host un-transposes)
            outsum = one.tile([OUTP, BL], F32, tag="outsum")
            nc.vector.tensor_reduce(
                outsum,
                pout.rearrange("p (b l) -> p b l", l=LO),
                axis=AX.X,
                op=ALU.add,
            )
            out_s = one.tile([OUTP, BL], F32, tag="out_s")
            nc.vector.tensor_scalar_add(out_s, outsum, fb)
            nc.sync.dma_start(out=outd, in_=out_s[0:OUT, :])

    nc.compile()
    return nc


@functools.lru_cache(maxsize=1)
def get_nc() -> bass.Bass:
    return build_nc()


def _swz(mat: np.ndarray) -> np.ndarray:
    """[H_in, X] -> [128, C, X]: partition-contiguous chunk swizzle."""
    return np.ascontiguousarray(
        mat.reshape(C, 128, -1).transpose(1, 0, 2)
    )


def make_in_maps(inputs: dict) -> list[dict]:
    art = np.ascontiguousarray(np.asarray(inputs["article_contexts"], np.float32))
    qc = np.ascontiguousarray(np.asarray(inputs["question_contexts"], np.float32))
    opt = np.ascontiguousarray(np.asarray(inputs["options_embeds"], np.float32))
    idx = np.asarray(inputs["answer_indices"]).astype(np.int64)

    def g(name):
        return np.asarray(inputs[name], np.float32)

    aQwT = _swz(np.ascontiguousarray(g("a_Qw").T))  # [128, C, H]
    aKwT = _swz(np.ascontiguousarray(g("a_Kw").T))
    wqk = np.stack([aQwT, aKwT], axis=1).astype(NP_F8)  # [128, 2, C, H]
    dKwT = _swz(np.ascontiguousarray(g("d_Kw").T)).astype(NP_BF16)
    # folded: aq -> options query projection
    Wqv = g("d_Qw") @ g("a_Vw")  # [H, H] (a_Vw maps h_in->h_out as aq = u @ a_Vw^T)
    qvwT = _swz(np.ascontiguousarray(Wqv.T.astype(np.float32))).astype(NP_BF16)
    bias_qv = g("d_Qw") @ g("a_Vb") + g("d_Qb") + g("d_Kb")  # [H]
    # folded: per-option final weights
    # feats[b,o,:] = u_d[b,o] @ d_Vw^T + d_Vb ; logits = sum_o feats[b,o] @ f_w[:,o]^T + f_b
    # => logits = sum_o u_d[b,o] @ (d_Vw^T @ f_w[:,o]^T) + (f_b + sum_o f_w[:,o] @ d_Vb)
    f_w = g("f_w")  # [OUT, 5H], flattened o-major
    dVwT = g("d_Vw").T  # [H_in, H_out]
    Ff = np.stack(
        [dVwT @ f_w[:, o * H : (o + 1) * H].T for o in range(NOPT)], axis=0
    )  # [o, H_in, OUT]
    fb_new = g("f_b") + sum(
        f_w[:, o * H : (o + 1) * H] @ g("d_Vb") for o in range(NOPT)
    )  # [OUT]
    fwT = np.zeros((128, NOPT, C, 8), np.float32)
    fwT[:, :, :, :OUT] = Ff.reshape(NOPT, C, 128, OUT).transpose(2, 0, 1, 3)
    fwT = fwT.astype(NP_BF16)

    def colvec(v):  # [H] -> [128, C] chunk-major
        return np.ascontiguousarray(np.asarray(v, np.float32).reshape(C, 128).T)

    vwa_col = colvec(g("a_vw").reshape(H))  # [128, C]
    vwaT = np.zeros((128, C2, 2, 16), np.float32)
    vwaT[:, :, :, 0] = vwa_col.reshape(128, C2, 2)
    vwaT = vwaT.astype(NP_F8)
    vwdT = colvec(g("d_vw").reshape(H)).astype(NP_BF16)
    qkbT = colvec(g("a_Qb") + g("a_Kb"))
    qvbT = colvec(bias_qv)

    # [B, H, LA] -> partition-swizzled [B, 128, C, LA]
    artT = (
        art.transpose(0, 2, 1)
        .reshape(B, C, 128, LA)
        .transpose(0, 2, 1, 3)
    )
    artT = np.ascontiguousarray(artT).astype(NP_F8)
    # [B, H, 5, LO] -> [B, 128, C, 5, LO]
    optT = (
        opt.transpose(0, 3, 1, 2)
        .reshape(B, C, 128, NOPT, LO)
        .transpose(0, 2, 1, 3, 4)
    )
    optT = np.ascontiguousarray(optT).astype(NP_BF16)
    onehot = np.zeros((B, LQ), np.float32)
    onehot[np.arange(B), idx] = 1.0
    onehot = onehot.astype(NP_F8)

    shared = dict(
        wqk=wqk, qvwT=qvwT, dKwT=dKwT,
        vwaT=vwaT, vwdT=vwdT, qkbT=qkbT, qvbT=qvbT,
        fwT=fwT,
        fb=np.ascontiguousarray(
            np.pad(fb_new.astype(np.float32), (0, 3)).reshape(OUTP, 1)
        ),
        ones1=np.ones((1, 128), np.float32),
    )
    qc8 = qc.astype(NP_F8)
    in_maps = []
    for r in range(NCORES):
        s = slice(r * BL, (r + 1) * BL)
        m = dict(shared)
        m["artT"] = artT[s]
        m["optT"] = optT[s]
        m["qc"] = qc8[s]
        m["oh"] = np.ascontiguousarray(onehot[s].T)
        in_maps.append(m)
    return in_maps


def run(inputs: dict, trace: bool = False, tmpdir=None):
    from concourse.bass_utils import run_bass_kernel_spmd

    nc = get_nc()
    in_maps = make_in_maps(inputs)
    res = run_bass_kernel_spmd(
        nc, in_maps, core_ids=list(range(NCORES)), trace=trace, tmpdir=tmpdir
    )
    out = np.concatenate(
        [res.results[r]["out"].T for r in range(NCORES)], axis=0
    )
    return out, res


def kernel(**inputs) -> np.ndarray:
    out, _ = run(inputs, trace=False)
    return out


# revision 20
# speedup vs baseline: 1.1643x; 1.0043x over previous
"""Trainium2 Bass kernel for nn_BaselineOut (article/option additive-attention MRC head).

Contract: kernel(**inputs) takes FULL unsharded inputs (numpy), returns FULL
[32, 5] float32 logits.  Internally: data-parallel over batch across 8 cores
(4 batch items per core), all params replicated.

Math notes (vs reference):
  - oqc gather is done as a one-hot matmul on device (host only encodes the
    int indices as a one-hot matrix - a layout/encoding transform).
  - V-projection is pulled out of the attention sum by linearity:
        sum_l softmax_l * (V @ Vw^T + Vb) = (sum_l softmax_l * V) @ Vw^T + Vb
    so the [B*L,H]x[H,H] V matmul collapses to a weighted sum over L plus a
    tiny [B,H]x[H,H] matmul.
  - Consecutive linear maps with no nonlinearity between are constant-folded
    on host (weight-weight products):
      * aq -> Qp_d: one matmul with Wqv = d_Qw @ a_Vw^T and a folded bias.
      * feats -> logits: per-option folded weights Ff_o = d_Vw^T @ f_w[:,o]^T.
  - softmax logit bias (vb) is dropped: softmax is shift-invariant.
  - exp is computed without max-subtraction: |logit| <= ||vw||_1 ~ 36, well
    inside fp32 exp range.
  - The article branch (Q/K projections, tanh dot, weighted sum) runs in
    fp8e4 with DoubleRow matmuls (2 k-chunks per instruction).  This is safe:
    the article attention output u only reaches the logits as a small
    additive query shift (std ~0.07) inside the option tanh (argument std
    ~1.4), so multi-percent noise in u damps to ~0.3% at the output.  The
    option branch K-projection stays bf16-or-better since its noise
    propagates ~1:1 into the logits.
  - The weighted V-sum is one fused DVE pass per h-chunk:
    scalar_tensor_tensor(T * scores_psum) with accum_out.
  - The option-branch normalization 1/sum is folded into the replicated
    score tile (sdn), and the final linear runs on the PE over the
    score-scaled option tile, producing logits^T; the host un-transposes.
  - Big tensors are host-swizzled so each SBUF partition's data is one long
    contiguous DRAM run (SWDGE throughput ~ run_length; wq/wk are fused into
    one 16KB-run tensor).  fp8/bf16 casting happens on host.  f32r must
    never be a DRAM I/O dtype (crashes NRT) - the one f32->f32r cast (ones)
    rides SWDGE.
  - The option K-projection (kpd) matmuls are issued before the uT/biasO
    chain and staged to SBUF via vector copies, so the PE fills the
    article->options dependency gap instead of idling.
"""

import functools
import sys

import numpy as np

sys.path.insert(0, "/opt/trn_rl_repo")

import ml_dtypes  # noqa: E402

import concourse.bass as bass  # noqa: E402
from concourse import bacc  # noqa: E402
import concourse.tile as tile  # noqa: E402
from concourse import mybir  # noqa: E402
from concourse.bass import ds, ts  # noqa: E402

B, LA, LQ, LO, H, OUT = 32, 2048, 64, 32, 1024, 5
NCORES = 8
BL = B // NCORES  # 4 batch items per core
NOPT = 5
F32 = mybir.dt.float32
F32R = mybir.dt.float32r
F8 = mybir.dt.float8e4
BF16 = mybir.dt.bfloat16
LT = 512  # article l-tile (free dim of the big matmuls)
NLT = LA // LT  # 4
C = H // 128  # 8 h-chunks
C2 = C // 2  # 4 h-chunk pairs (DoubleRow)
BO = BL * NOPT  # 20 (b, option) pairs per core
AF = mybir.ActivationFunctionType
ALU = mybir.AluOpType
AX = mybir.AxisListType
DR = mybir.MatmulPerfMode.DoubleRow
OUTP = 8  # final-linear out dim padded even
HALF = 2 * NOPT * LO  # 320 option columns (2 batch items)
BOH = BO // 2  # 10 (b,o) pairs per half
NP_F8 = ml_dtypes.float8_e4m3
NP_BF16 = ml_dtypes.bfloat16


def build_nc() -> bass.Bass:
    nc = bacc.Bacc("TRN2", target_bir_lowering=False, debug=False)

    # ---- DRAM I/O (per-core shard; names are the in_map keys) ----
    # Big tensors are pre-swizzled on host to [128(partition), ...contiguous].
    artT = nc.dram_tensor("artT", [BL, 128, C, LA], F8, kind="ExternalInput").ap()
    optT = nc.dram_tensor(
        "optT", [BL, 128, C, NOPT, LO], BF16, kind="ExternalInput"
    ).ap()
    qcd = nc.dram_tensor("qc", [BL, LQ, H], F8, kind="ExternalInput").ap()
    ohd = nc.dram_tensor("oh", [LQ, BL], F8, kind="ExternalInput").ap()
    wqkd = nc.dram_tensor("wqk", [128, 2, C, H], F8, kind="ExternalInput").ap()
    wQV = nc.dram_tensor("qvwT", [128, C, H], BF16, kind="ExternalInput").ap()
    wKd = nc.dram_tensor("dKwT", [128, C, H], BF16, kind="ExternalInput").ap()
    vwad = nc.dram_tensor("vwaT", [128, C2, 2, 16], F8, kind="ExternalInput").ap()
    vwdd = nc.dram_tensor("vwdT", [128, C], BF16, kind="ExternalInput").ap()
    qkbd = nc.dram_tensor("qkbT", [128, C], F32, kind="ExternalInput").ap()
    qvbd = nc.dram_tensor("qvbT", [128, C], F32, kind="ExternalInput").ap()
    fwd = nc.dram_tensor("fwT", [128, NOPT, C, OUTP], BF16, kind="ExternalInput").ap()
    fbd = nc.dram_tensor("fb", [OUTP, 1], F32, kind="ExternalInput").ap()
    onesd = nc.dram_tensor("ones1", [1, 128], F32, kind="ExternalInput").ap()
    outd = nc.dram_tensor("out", [OUT, BL], F32, kind="ExternalOutput").ap()

    with (
        tile.TileContext(nc) as tc,
        nc.allow_low_precision(reason="fp8/bf16 article branch; PE accums fp32"),
    ):
        with (
            tc.tile_pool(name="stream", bufs=1) as stream,
            tc.tile_pool(name="art", bufs=2) as artp,
            tc.tile_pool(name="wbig", bufs=1) as wbig,
            tc.tile_pool(name="mpool", bufs=3) as mpool,
            tc.tile_pool(name="spool", bufs=2) as spool,
            tc.tile_pool(name="rdpool", bufs=1) as rdpool,
            tc.tile_pool(name="ubuf", bufs=2) as ubuf,
            tc.tile_pool(name="scratch", bufs=1) as scratch,
            tc.tile_pool(name="one", bufs=1) as one,
            tc.tile_pool(name="pacc", bufs=3, space="PSUM") as pacc,
            tc.tile_pool(name="pprep", bufs=2, space="PSUM") as pprep,
            tc.tile_pool(name="prow", bufs=2, space="PSUM") as prow,
            tc.tile_pool(name="psml", bufs=1, space="PSUM") as psml,
        ):
            # ---------- SWDGE (gpsimd): the latency-critical byte loads ----
            # Order: head tensors (oqc/biasA inputs) first so the PE head
            # overlaps the big article loads.
            qct = stream.tile([LQ, BL, H], F8, tag="qct")
            for b in range(BL):
                nc.gpsimd.dma_start(out=qct[:, b, :], in_=qcd[b])
            oht = one.tile([LQ, BL], F8, tag="oht")
            nc.gpsimd.dma_start(out=oht, in_=ohd)
            # wqk rides HWDGE so its descriptor generation overlaps art0's
            # on SWDGE (~90ns/16KB packet each, ~11.5us serial otherwise)
            wqk = wbig.tile([128, 2, C, H], F8, tag="wqk")
            nc.sync.dma_start(out=wqk, in_=wqkd)
            wq = wqk[:, 0]
            wk = wqk[:, 1]
            art = [artp.tile([128, C, LA], F8, tag="art", name=f"art{b}")
                   for b in range(BL)]
            nc.gpsimd.dma_start(out=art[0], in_=artT[0])
            ones = one.tile([1, 128], F32R, tag="ones")
            nc.gpsimd.dma_start(out=ones, in_=onesd)
            nc.gpsimd.dma_start(out=art[1], in_=artT[1])

            # ---------- HWDGE (sync): consts + option-branch weights ----------
            # v-dot weights, padded to a 16B k-pair stride (dual-fp8
            # ldweights ISA restriction); column 0 is the real vw value.
            vwa = one.tile([128, C2, 2, 16], F8, tag="vwa")
            nc.sync.dma_start(out=vwa, in_=vwad)
            vwd = one.tile([128, C], BF16, tag="vwd")
            nc.sync.dma_start(out=vwd, in_=vwdd)
            qkb = one.tile([128, C], F32, tag="qkb")
            nc.sync.dma_start(out=qkb, in_=qkbd)
            qvb = one.tile([128, C], F32, tag="qvb")
            nc.sync.dma_start(out=qvb, in_=qvbd)
            fw = one.tile([128, NOPT, C, OUTP], BF16, tag="fw")
            nc.sync.dma_start(out=fw, in_=fwd)
            fb = one.tile([OUTP, 1], F32, tag="fb")
            nc.sync.dma_start(out=fb, in_=fbd)
            wqv = wbig.tile([128, C, H], BF16, tag="w", bufs=2)
            wdk = wbig.tile([128, C, H], BF16, tag="w", bufs=2)
            nc.sync.dma_start(out=wqv, in_=wQV)
            nc.sync.dma_start(out=wdk, in_=wKd)
            OT = stream.tile([128, C, BL, NOPT, LO], BF16, tag="ot")
            for b in range(BL):
                nc.sync.dma_start(out=OT[:, :, b], in_=optT[b])

            # ---------- gather oqc via one-hot matmul ----------
            oqcT = one.tile([128, C, BL], F8, tag="oqcT")
            for c in range(C):
                po = pacc.tile([128, BL], F32, tag="acc")
                for b in range(BL):
                    nc.tensor.matmul(
                        po[:, b : b + 1],
                        lhsT=qct[:, b, ts(c, 128)],
                        rhs=oht[:, b : b + 1],
                        start=True,
                        stop=True,
                    )
                nc.vector.tensor_copy(oqcT[:, c, :], po)

            # ---------- Qp^T = aQw @ oqc^T ; article tanh bias ----------
            biasA = one.tile([128, C, BL], F32, tag="biasA")
            for co in range(C):
                pq = pacc.tile([128, BL], F32, tag="acc")
                for ci in range(C):
                    nc.tensor.matmul(
                        pq,
                        lhsT=wq[:, ci, ts(co, 128)],
                        rhs=oqcT[:, ci, :],
                        start=(ci == 0),
                        stop=(ci == C - 1),
                    )
                nc.vector.tensor_scalar_add(biasA[:, co, :], pq, qkb[:, co : co + 1])

            # ---------- article branch ----------
            # Per (b, lt): fp8 DoubleRow K-projection -> tanh (+bias) -> fp8
            # DoubleRow v-dot -> exp -> PE-replicated scores -> fused
            # multiply+reduce weighted V-sum on DVE.
            s_sums = one.tile([1, BL, NLT], F32, tag="s_sums")
            uTun = one.tile([128, C, BL], F32, tag="uTun")
            for b in range(BL):
                if b + 2 <= BL - 1:
                    nc.gpsimd.dma_start(out=art[b + 2], in_=artT[b + 2])
                T = art[b]
                upart = ubuf.tile([128, C, NLT], F32, tag="upart")
                for lt in range(NLT):
                    lg = prow.tile([2, LT], F32, tag="lg")
                    for cop in range(C2):
                        mt2 = mpool.tile([128, 2, LT], F8, tag="mt")
                        for half in range(2):
                            co = 2 * cop + half
                            kp = pacc.tile([128, LT], F32, tag="acc")
                            for ci2 in range(C2):
                                nc.tensor.matmul(
                                    kp,
                                    lhsT=wk[:, 2 * ci2 : 2 * ci2 + 2, ts(co, 128)],
                                    rhs=T[:, 2 * ci2 : 2 * ci2 + 2, ds(lt * LT, LT)],
                                    start=(ci2 == 0),
                                    stop=(ci2 == C2 - 1),
                                    perf_mode=DR,
                                )
                            nc.scalar.activation(
                                mt2[:, half, :], kp, AF.Tanh,
                                bias=biasA[:, co, b : b + 1],
                            )
                        nc.tensor.matmul(
                            lg,
                            lhsT=vwa[:, cop, :, 0:2],
                            rhs=mt2,
                            start=(cop == 0),
                            stop=(cop == C2 - 1),
                            perf_mode=DR,
                        )
                    st = spool.tile([1, LT], F32R, tag="st")
                    nc.scalar.activation(
                        st, lg[0:1, :], AF.Exp, accum_out=s_sums[:, b, lt : lt + 1]
                    )
                    # replicate s~ across partitions: ones^T (x) st via PE
                    prep = pprep.tile([128, LT], F32, tag="prep")
                    nc.tensor.matmul(prep, lhsT=ones, rhs=st, start=True, stop=True)
                    # fused weighted V-sum: upart[:,c,lt] = sum_l T*s, one pass
                    for c in range(C):
                        scr = scratch.tile([128, LT], F32, tag="scr")
                        nc.vector.scalar_tensor_tensor(
                            scr,
                            T[:, c, ds(lt * LT, LT)],
                            1.0,
                            prep,
                            op0=ALU.mult,
                            op1=ALU.mult,
                            accum_out=upart[:, c, lt : lt + 1],
                        )
                # sum the NLT partial weighted sums -> unnormalized u^T
                nc.vector.tensor_reduce(
                    uTun[:, :, b : b + 1], upart, axis=AX.X, op=ALU.add
                )

            # ---------- options K-projection, issued before the biasO chain
            # so the PE fills the article->options dependency gap; results
            # staged to SBUF via vector copies (PSUM ring stays small).
            kpds = stream.tile([128, C, 2, HALF], F32, tag="kpds")
            for co in range(C):
                for h in range(2):
                    kpd = pacc.tile([128, HALF], F32, tag="acc")
                    for ci in range(C):
                        nc.tensor.matmul(
                            kpd,
                            lhsT=wdk[:, ci, ts(co, 128)],
                            rhs=OT[:, ci, ds(2 * h, 2)],
                            start=(ci == 0),
                            stop=(ci == C - 1),
                        )
                    nc.vector.tensor_copy(kpds[:, co, h], kpd)

            # normalization factors: 1/sum(exp) per b, replicated to 128 parts
            ssb = one.tile([1, BL], F32, tag="ssb")
            nc.vector.tensor_reduce(ssb, s_sums, axis=AX.X, op=ALU.add)
            psb = psml.tile([128, BL], F32, tag="sml")
            nc.tensor.matmul(
                psb, lhsT=ones.bitcast(F32), rhs=ssb, start=True, stop=True
            )
            rs_rep = one.tile([128, BL], F32, tag="rs_rep")
            nc.vector.reciprocal(rs_rep, psb)

            uT = one.tile([128, C, BL], BF16, tag="uT")
            for b in range(BL):
                nc.vector.tensor_scalar_mul(
                    uT[:, :, b], uTun[:, :, b], rs_rep[:, b : b + 1]
                )

            # ---------- option tanh bias via folded Wqv = d_Qw a_Vw^T ----------
            biasO = one.tile([128, C, BL], F32, tag="biasO")
            for co in range(C):
                pq2 = pacc.tile([128, BL], F32, tag="acc")
                for ci in range(C):
                    nc.tensor.matmul(
                        pq2,
                        lhsT=wqv[:, ci, ts(co, 128)],
                        rhs=uT[:, ci, :],
                        start=(ci == 0),
                        stop=(ci == C - 1),
                    )
                nc.vector.tensor_scalar_add(biasO[:, co, :], pq2, qvb[:, co : co + 1])

            # ---------- options branch (tanh from staged kpds) ----------
            # biasO is broadcast-added into kpds on the vector engine (it
            # varies per b, so it can't be an activation bias for a merged
            # tile), then one big tanh per co on scalar; vector runs one co
            # ahead of scalar.
            mdt = stream.tile([128, C, BL, NOPT, LO], BF16, tag="mdt")
            BX = NOPT * LO  # 160 columns per batch item
            for co in range(C):
                kv = kpds[:, co].rearrange("p h (b x) -> p h b x", b=2)
                bv = (
                    biasO[:, co, :]
                    .rearrange("p (h b) -> p h b", b=2)
                    .unsqueeze(3)
                    .broadcast_to((128, 2, 2, BX))
                )
                nc.vector.scalar_tensor_tensor(
                    kv, kv, 0.0, bv, op0=ALU.add, op1=ALU.add
                )
                nc.scalar.activation(
                    mdt[:, co].rearrange("p b o l -> p (b o l)"),
                    kpds[:, co].rearrange("p h x -> p (h x)"),
                    AF.Tanh,
                )

            s_d = one.tile([1, BO * LO], F32R, tag="s_d")
            for h in range(2):
                lgd = prow.tile([1, HALF], F32, tag="lg")
                for co in range(C):
                    nc.tensor.matmul(
                        lgd,
                        lhsT=vwd[:, co : co + 1],
                        rhs=mdt[:, co, ds(2 * h, 2)],
                        start=(co == 0),
                        stop=(co == C - 1),
                    )
                nc.scalar.activation(s_d[:, ds(h * HALF, HALF)], lgd, AF.Exp)

            sums_d = one.tile([1, BO], F32, tag="sums_d")
            nc.vector.tensor_reduce(
                sums_d,
                s_d.bitcast(F32).rearrange("p (bo l) -> p bo l", l=LO),
                axis=AX.X,
                op=ALU.add,
            )
            rec_d = one.tile([1, BO], F32, tag="rec_d")
            nc.vector.reciprocal(rec_d, sums_d)
            prec = psml.tile([128, BO], F32, tag="sml")
            nc.tensor.matmul(
                prec, lhsT=ones.bitcast(F32), rhs=rec_d, start=True, stop=True
            )
            rec_rep = one.tile([128, BO], F32, tag="rec_rep")
            nc.scalar.copy(rec_rep, prec)

            # replicate exp scores with 1/sum folded in: sdn = s_d * rec
            sdn = rdpool.tile([128, BO, LO], BF16, tag="sdn")
            for h in range(2):
                prepd = pprep.tile([128, HALF], F32, tag="prep")
                nc.tensor.matmul(
                    prepd,
                    lhsT=ones,
                    rhs=s_d[:, ds(h * HALF, HALF)],
                    start=True,
                    stop=True,
                )
                nc.vector.scalar_tensor_tensor(
                    sdn[:, ds(h * BOH, BOH)],
                    rec_rep[:, ds(h * BOH, BOH)]
                    .unsqueeze(2)
                    .broadcast_to((128, BOH, LO)),
                    1.0,
                    prepd.rearrange("p (bo l) -> p bo l", l=LO),
                    op0=ALU.mult,
                    op1=ALU.mult,
                )

            # final linear on the PE over the score-scaled option tile:
            # logitsT[j, (b,l)] += fw[:,o,c,:].T @ (OT[:,c] * sdn)[:, :, o, :]
            OTf = OT.rearrange("p c b o l -> p c (b o) l")
            pout = psml.tile([OUTP, BL * LO], F32, tag="sml")
            for c in range(C):
                scrd = scratch.tile([128, BO, LO], BF16, tag="scrd", bufs=2)
                nc.vector.tensor_mul(scrd, OTf[:, c], sdn)
                sv = scrd.rearrange("p (b o) l -> p b o l", o=NOPT)
                for o in range(NOPT):
                    nc.tensor.matmul(
                        pout,
                        lhsT=fw[:, o, c, :],
                        rhs=sv[:, :, o, :],
                        start=(c == 0 and o == 0),
                        stop=(c == C - 1 and o == NOPT - 1),
                    )
            # reduce over l, add bias, store logits^T (host un-transposes)
            outsum = one.tile([OUTP, BL], F32, tag="outsum")
            nc.vector.tensor_reduce(
                outsum,
                pout.rearrange("p (b l) -> p b l", l=LO),
                axis=AX.X,
                op=ALU.add,
            )
            out_s = one.tile([OUTP, BL], F32, tag="out_s")
            nc.vector.tensor_scalar_add(out_s, outsum, fb)
            nc.sync.dma_start(out=outd, in_=out_s[0:OUT, :])

    nc.compile()
    return nc


@functools.lru_cache(maxsize=1)
def get_nc() -> bass.Bass:
    return build_nc()


def _swz(mat: np.ndarray) -> np.ndarray:
    """[H_in, X] -> [128, C, X]: partition-contiguous chunk swizzle."""
    return np.ascontiguousarray(
        mat.reshape(C, 128, -1).transpose(1, 0, 2)
    )


def make_in_maps(inputs: dict) -> list[dict]:
    art = np.ascontiguousarray(np.asarray(inputs["article_contexts"], np.float32))
    qc = np.ascontiguousarray(np.asarray(inputs["question_contexts"], np.float32))
    opt = np.ascontiguousarray(np.asarray(inputs["options_embeds"], np.float32))
    idx = np.asarray(inputs["answer_indices"]).astype(np.int64)

    def g(name):
        return np.asarray(inputs[name], np.float32)

    aQwT = _swz(np.ascontiguousarray(g("a_Qw").T))  # [128, C, H]
    aKwT = _swz(np.ascontiguousarray(g("a_Kw").T))
    wqk = np.stack([aQwT, aKwT], axis=1).astype(NP_F8)  # [128, 2, C, H]
    dKwT = _swz(np.ascontiguousarray(g("d_Kw").T)).astype(NP_BF16)
    # folded: aq -> options query projection
    Wqv = g("d_Qw") @ g("a_Vw")  # [H, H] (a_Vw maps h_in->h_out as aq = u @ a_Vw^T)
    qvwT = _swz(np.ascontiguousarray(Wqv.T.astype(np.float32))).astype(NP_BF16)
    bias_qv = g("d_Qw") @ g("a_Vb") + g("d_Qb") + g("d_Kb")  # [H]
    # folded: per-option final weights
    # feats[b,o,:] = u_d[b,o] @ d_Vw^T + d_Vb ; logits = sum_o feats[b,o] @ f_w[:,o]^T + f_b
    # => logits = sum_o u_d[b,o] @ (d_Vw^T @ f_w[:,o]^T) + (f_b + sum_o f_w[:,o] @ d_Vb)
    f_w = g("f_w")  # [OUT, 5H], flattened o-major
    dVwT = g("d_Vw").T  # [H_in, H_out]
    Ff = np.stack(
        [dVwT @ f_w[:, o * H : (o + 1) * H].T for o in range(NOPT)], axis=0
    )  # [o, H_in, OUT]
    fb_new = g("f_b") + sum(
        f_w[:, o * H : (o + 1) * H] @ g("d_Vb") for o in range(NOPT)
    )  # [OUT]
    fwT = np.zeros((128, NOPT, C, 8), np.float32)
    fwT[:, :, :, :OUT] = Ff.reshape(NOPT, C, 128, OUT).transpose(2, 0, 1, 3)
    fwT = fwT.astype(NP_BF16)

    def colvec(v):  # [H] -> [128, C] chunk-major
        return np.ascontiguousarray(np.asarray(v, np.float32).reshape(C, 128).T)

    vwa_col = colvec(g("a_vw").reshape(H))  # [128, C]
    vwaT = np.zeros((128, C2, 2, 16), np.float32)
    vwaT[:, :, :, 0] = vwa_col.reshape(128, C2, 2)
    vwaT = vwaT.astype(NP_F8)
    vwdT = colvec(g("d_vw").reshape(H)).astype(NP_BF16)
    qkbT = colvec(g("a_Qb") + g("a_Kb"))
    qvbT = colvec(bias_qv)

    # [B, H, LA] -> partition-swizzled [B, 128, C, LA]
    artT = (
        art.transpose(0, 2, 1)
        .reshape(B, C, 128, LA)
        .transpose(0, 2, 1, 3)
    )
    artT = np.ascontiguousarray(artT).astype(NP_F8)
    # [B, H, 5, LO] -> [B, 128, C, 5, LO]
    optT = (
        opt.transpose(0, 3, 1, 2)
        .reshape(B, C, 128, NOPT, LO)
        .transpose(0, 2, 1, 3, 4)
    )
    optT = np.ascontiguousarray(optT).astype(NP_BF16)
    onehot = np.zeros((B, LQ), np.float32)
    onehot[np.arange(B), idx] = 1.0
    onehot = onehot.astype(NP_F8)

    shared = dict(
        wqk=wqk, qvwT=qvwT, dKwT=dKwT,
        vwaT=vwaT, vwdT=vwdT, qkbT=qkbT, qvbT=qvbT,
        fwT=fwT,
        fb=np.ascontiguousarray(
            np.pad(fb_new.astype(np.float32), (0, 3)).reshape(OUTP, 1)
        ),
        ones1=np.ones((1, 128), np.float32),
    )
    qc8 = qc.astype(NP_F8)
    in_maps = []
    for r in range(NCORES):
        s = slice(r * BL, (r + 1) * BL)
        m = dict(shared)
        m["artT"] = artT[s]
        m["optT"] = optT[s]
        m["qc"] = qc8[s]
        m["oh"] = np.ascontiguousarray(onehot[s].T)
        in_maps.append(m)
    return in_maps


def run(inputs: dict, trace: bool = False, tmpdir=None):
    from concourse.bass_utils import run_bass_kernel_spmd

    nc = get_nc()
    in_maps = make_in_maps(inputs)
    res = run_bass_kernel_spmd(
        nc, in_maps, core_ids=list(range(NCORES)), trace=trace, tmpdir=tmpdir
    )
    out = np.concatenate(
        [res.results[r]["out"].T for r in range(NCORES)], axis=0
    )
    return out, res


def kernel(**inputs) -> np.ndarray:
    out, _ = run(inputs, trace=False)
    return out
